# revision 15
# baseline (speedup 1.0000x reference)
"""Trainium2 Bass kernel: causal multi-head attention with RoPE.

Reference computation (B=2, T=2048, C=2048, H=16, D=128, fp32):
    q/k/v = hs @ {q,k,v}_w^T ; RoPE(q), RoPE(k)
    out   = softmax(causal(q k^T / sqrt(D))) v @ o_w^T

Sharding: tensor-parallel over heads — each of the 8 cores owns 2 heads.
Each core computes its heads' projections + attention and a partial output
projection; the host sums the 8 partials.

Per-core device pipeline (all matmuls in float32r = full-rate fp32):
  A) stream hs^T chunks; qT/kT in [d, t] layout (per-window tiles so
     later stages only wait on the exact window they read), v in [t, d]
     layout; RoPE (rotate_half as a constant +-1 permutation matmul +
     cos/sin elementwise) interleaved per pair of chunks.
  B) scores computed TRANSPOSED [tk, tq]; exp on ACT with 1/sqrt(D)
     folded into the activation scale; causal masking as a 0/1 multiply
     on block-diagonal tiles only; softmax denominator via an M=1
     all-ones matmul accumulated alongside PV; unnormalized attnT
     overwrites the spent q window tiles.  Per head: reciprocal of the
     denominators via exp(-ln(x)) (one ACT table-set switch pair per
     head, overlapped with the next head's attention), gpsimd
     partition-broadcast, normalize in place.
  C) output projection interleaved with the last head's normalizes;
     partial [t, c] tiles to DRAM.
"""

import math
import sys

if "/opt/trn_rl_repo" not in sys.path:
    sys.path.insert(0, "/opt/trn_rl_repo")

import numpy as np

import concourse.bass as bass
import concourse.mybir as mybir
import concourse.tile as tile
from concourse import bacc, bass_utils

F32 = mybir.dt.float32
F32R = mybir.dt.float32r
AF = mybir.ActivationFunctionType
MULT = mybir.AluOpType.mult
ADD = mybir.AluOpType.add

B = 2
C = 2048
H = 16
D = 128
N_CORES = 8
HPC = H // N_CORES  # heads per core
DPC = HPC * D  # channels per core (256)
ROPE_BASE = 10000.0
P = 128  # partitions
TQW = 512  # tq window (matmul free dim)
TCH = 256  # hs^T chunk width in t


def _build_nc(T: int = 2048):
    """Build the per-core Bass program (SPMD: same program, per-core data)."""
    KT = C // P  # 16 k-tiles over the contraction dim c
    n_ch = T // TCH  # hs chunks per batch
    n_w = T // TQW  # tq windows per (b, h)
    cpw = TQW // TCH  # chunks per window
    scale = 1.0 / math.sqrt(D)

    nc = bacc.Bacc(trn_type="TRN2", target_bir_lowering=False, debug=False)

    hst = nc.dram_tensor("hst", [B, C, T], F32R, kind="ExternalInput").ap()
    wq = nc.dram_tensor("wq_t", [C, DPC], F32R, kind="ExternalInput").ap()
    wk = nc.dram_tensor("wk_t", [C, DPC], F32R, kind="ExternalInput").ap()
    wv = nc.dram_tensor("wv_t", [C, DPC], F32R, kind="ExternalInput").ap()
    ow = nc.dram_tensor("ow_t", [DPC, C], F32R, kind="ExternalInput").ap()
    cos_d = nc.dram_tensor("cos_t", [D, T], F32, kind="ExternalInput").ap()
    sin_d = nc.dram_tensor("sin_t", [D, T], F32, kind="ExternalInput").ap()
    rp_d = nc.dram_tensor("rperm", [D, D], F32R, kind="ExternalInput").ap()
    ones_d = nc.dram_tensor("ones", [P, 1], F32R, kind="ExternalInput").ap()
    msk_d = nc.dram_tensor("masks", [TQW // P, P, TQW], F32, kind="ExternalInput").ap()
    out_d = nc.dram_tensor("out_p", [B, T, C], F32, kind="ExternalOutput").ap()

    with tile.TileContext(nc) as tc:
        with (
            tc.tile_pool(name="consts", bufs=1) as consts,
            tc.tile_pool(name="hst", bufs=2) as hstp,
            tc.tile_pool(name="qkv", bufs=1) as qkvp,
            tc.tile_pool(name="exp", bufs=6) as expp,
            tc.tile_pool(name="small", bufs=2) as smallp,
            tc.tile_pool(name="outp", bufs=3) as outp,
            tc.tile_pool(name="ps", bufs=8, space="PSUM") as ps,
        ):
            # ---- resident constants -------------------------------------
            wq_sb = consts.tile([P, KT, DPC], F32R, tag="wq")
            wk_sb = consts.tile([P, KT, DPC], F32R, tag="wk")
            wv_sb = consts.tile([P, KT, DPC], F32R, tag="wv")
            ow_sb = consts.tile([P, HPC, C], F32R, tag="ow")
            cos_sb = consts.tile([D, T], F32, tag="cos")
            sin_sb = consts.tile([D, T], F32, tag="sin")
            msk_sb = consts.tile([P, TQW // P, TQW], F32, tag="msk")
            ones_sb = consts.tile([P, 1], F32R, tag="ones")
            rp_sb = consts.tile([D, D], F32R, tag="rp")
            for w_sb, w_d in ((wq_sb, wq), (wk_sb, wk), (wv_sb, wv)):
                nc.sync.dma_start(w_sb[:], w_d.rearrange("(ko p) d -> p ko d", p=P))
            nc.sync.dma_start(ow_sb[:], ow.rearrange("(h p) c -> p h c", p=P))
            nc.sync.dma_start(cos_sb[:], cos_d)
            nc.sync.dma_start(sin_sb[:], sin_d)
            nc.sync.dma_start(msk_sb[:], msk_d.rearrange("o p x -> p o x"))
            nc.sync.dma_start(ones_sb[:], ones_d)
            nc.sync.dma_start(rp_sb[:], rp_d)

            for b in range(B):
                # Per-window q/k tiles: fine-grained deps (a window's
                # consumers only wait on that window's producers).
                q_t = [
                    [qkvp.tile([P, TQW], F32R, tag=f"q{h}w{w}", name=f"q{h}w{w}") for w in range(n_w)]
                    for h in range(HPC)
                ]
                k_t = [
                    [qkvp.tile([P, TQW], F32R, tag=f"k{h}w{w}", name=f"k{h}w{w}") for w in range(n_w)]
                    for h in range(HPC)
                ]
                v_sb = qkvp.tile([P, T // P, DPC], F32R, tag="v")
                # Broadcast denominator tiles: row 0 receives the den row,
                # gpsimd replicates it across partitions during attention.
                bc_t = [
                    [qkvp.tile([P, TQW], F32, tag=f"bc{h}w{w}", name=f"bc{h}w{w}") for w in range(n_w)]
                    for h in range(HPC)
                ]

                # ---- phase A: projections + RoPE ------------------------
                def rope(w, b=b):
                    sl = bass.ts(w, TQW)
                    for h in range(HPC):
                        for x_t in (q_t, k_t):
                            x = x_t[h][w]
                            rh = ps.tile([P, TQW], F32, tag="ps")
                            nc.tensor.matmul(
                                rh[:], rp_sb[:], x[:], start=True, stop=True
                            )
                            t1 = smallp.tile([P, TQW], F32, tag="t1")
                            nc.vector.tensor_tensor(
                                t1[:], x[:].bitcast(F32), cos_sb[:, sl], op=MULT
                            )
                            nc.vector.tensor_tensor(rh[:], rh[:], sin_sb[:, sl], op=MULT)
                            nc.vector.tensor_tensor(x[:], t1[:], rh[:], op=ADD)

                ctx_a = nc.named_scope(f"A{b}"); ctx_a.__enter__()
                for ch in range(n_ch):
                    w, cw = divmod(ch, cpw)
                    slc = bass.ts(cw, TCH)
                    hst_t = hstp.tile([P, KT, TCH], F32R, tag="hst")
                    nc.sync.dma_start(
                        hst_t[:],
                        hst[b].rearrange("(ko p) t -> p ko t", p=P)[
                            :, :, bass.ts(ch, TCH)
                        ],
                    )
                    for h in range(HPC):
                        for x_t, w_sb in ((q_t, wq_sb), (k_t, wk_sb)):
                            pt = ps.tile([P, TQW], F32, tag="ps")
                            for k in range(KT):
                                nc.tensor.matmul(
                                    pt[:, :TCH],
                                    w_sb[:, k, bass.ts(h, D)],
                                    hst_t[:, k, :],
                                    start=(k == 0),
                                    stop=(k == KT - 1),
                                )
                            nc.scalar.activation(
                                x_t[h][w][:, slc], pt[:, :TCH], AF.Copy
                            )
                    for sub in range(TCH // P):
                        pt = ps.tile([P, TQW], F32, tag="ps")
                        for k in range(KT):
                            nc.tensor.matmul(
                                pt[:, :DPC],
                                hst_t[:, k, bass.ts(sub, P)],
                                wv_sb[:, k, :],
                                start=(k == 0),
                                stop=(k == KT - 1),
                            )
                        nc.scalar.activation(
                            v_sb[:, ch * (TCH // P) + sub, :], pt[:, :DPC], AF.Copy
                        )
                    # RoPE as soon as a full tq window of q/k is projected.
                    if cw == cpw - 1:
                        rope(w)

                ctx_a.__exit__(None, None, None)
                # ---- phase B: attention (unnormalized) ------------------
                def attend(h):
                    for w in range(n_w):
                        ntk = (w + 1) * (TQW // P)

                        def qk_exp(i, h=h, w=w):
                            st = ps.tile([P, TQW], F32, tag="ps")
                            nc.tensor.matmul(
                                st[:],
                                k_t[h][i // (TQW // P)][:, bass.ts(i % (TQW // P), P)],
                                q_t[h][w][:],
                                start=True,
                                stop=True,
                            )
                            e = expp.tile([P, TQW], F32R, tag="exp")
                            nc.scalar.activation(e[:], st[:], AF.Exp, scale=scale)
                            off = i - w * (TQW // P)
                            if off >= 0:
                                nc.vector.tensor_tensor(
                                    e[:], e[:].bitcast(F32), msk_sb[:, off, :], op=MULT
                                )
                            return e

                        fifo = [qk_exp(0)]
                        if ntk > 1:
                            fifo.append(qk_exp(1))
                        pv = ps.tile([P, TQW], F32, tag="ps")
                        den = ps.tile([P, TQW], F32, tag="ps")
                        for i in range(ntk):
                            if i + 2 < ntk:
                                fifo.append(qk_exp(i + 2))
                            e = fifo.pop(0)
                            nc.tensor.matmul(
                                pv[:],
                                v_sb[:, i, bass.ts(h, D)],
                                e[:],
                                start=(i == 0),
                                stop=(i == ntk - 1),
                            )
                            nc.tensor.matmul(
                                den[:1, :],
                                ones_sb[:],
                                e[:],
                                start=(i == 0),
                                stop=(i == ntk - 1),
                            )
                        # stash unnormalized attnT into the spent q window;
                        # den row -> bc tile row 0, replicate on gpsimd.
                        nc.vector.tensor_copy(q_t[h][w][:], pv[:])
                        bc = bc_t[h][w]
                        nc.vector.tensor_copy(bc[:1, :], den[:1, :])
                        nc.gpsimd.partition_broadcast(bc[:], bc[:1, :])

                def recip(h):
                    # exp(-ln(x)) in place on the broadcast tiles; Ln+Exp
                    # batched so the ACT table set switches only once.
                    for w in range(n_w):
                        nc.scalar.activation(bc_t[h][w][:], bc_t[h][w][:], AF.Ln)
                    for w in range(n_w):
                        nc.scalar.activation(
                            bc_t[h][w][:], bc_t[h][w][:], AF.Exp, scale=-1.0
                        )

                def normalize(h, w):
                    nc.vector.tensor_tensor(
                        q_t[h][w][:],
                        q_t[h][w][:].bitcast(F32),
                        bc_t[h][w][:],
                        op=MULT,
                    )

                with nc.named_scope(f"B{b}"):
                    attend(0)
                    attend(1)
                    recip(0)
                    recip(1)

                # ---- phase C: output projection, interleaved with the
                # normalizes ----------------------------------------------
                ctx_c = nc.named_scope(f"C{b}"); ctx_c.__enter__()
                for w in range(n_w):
                    normalize(0, w)
                    normalize(1, w)
                    for m in range(w * (TQW // P), (w + 1) * (TQW // P)):
                        for n in range(C // TQW):
                            po = ps.tile([P, TQW], F32, tag="ps")
                            for h in range(HPC):
                                nc.tensor.matmul(
                                    po[:],
                                    q_t[h][m // (TQW // P)][
                                        :, bass.ts(m % (TQW // P), P)
                                    ],
                                    ow_sb[:, h, bass.ts(n, TQW)],
                                    start=(h == 0),
                                    stop=(h == HPC - 1),
                                )
                            o_t = outp.tile([P, TQW], F32, tag="o")
                            nc.any.tensor_copy(o_t[:], po[:])
                            nc.sync.dma_start(
                                out_d[b, bass.ts(m, P), bass.ts(n, TQW)], o_t[:]
                            )
                ctx_c.__exit__(None, None, None)

    nc.compile()
    return nc


def _host_prep(hidden_states, q_w, k_w, v_w, o_w):
    """Build the 8 per-core input maps (and shared constant tensors)."""
    T = hidden_states.shape[1]
    f32 = np.float32

    hst = np.ascontiguousarray(hidden_states.transpose(0, 2, 1)).astype(f32, copy=False)

    inv_freq = 1.0 / (ROPE_BASE ** (np.arange(0, D, 2, dtype=np.float64) / D))
    t_ar = np.arange(T, dtype=np.float64)
    freqs = t_ar[:, None] * inv_freq[None, :]  # [T, D/2]
    cos_td = np.concatenate([np.cos(freqs), np.cos(freqs)], axis=-1)  # [T, D]
    sin_td = np.concatenate([np.sin(freqs), np.sin(freqs)], axis=-1)
    cos_t = np.ascontiguousarray(cos_td.T).astype(f32)  # [D, T]
    sin_t = np.ascontiguousarray(sin_td.T).astype(f32)

    # rotate_half as a matmul: rh = R @ x ; rperm = R^T (lhsT operand).
    rperm = np.zeros((D, D), dtype=f32)
    half = D // 2
    for j in range(half):
        rperm[2 * j + 1, j] = -1.0
    for j in range(half, D):
        rperm[2 * (j - half), j] = 1.0

    ones = np.ones((P, 1), dtype=f32)

    n_off = TQW // P
    masks = np.zeros((n_off, P, TQW), dtype=f32)
    y = np.arange(P)[:, None]
    x = np.arange(TQW)[None, :]
    for o in range(n_off):
        masks[o] = (x >= P * o + y).astype(f32)

    in_maps = []
    for c in range(N_CORES):
        rs, re = c * DPC, (c + 1) * DPC
        in_maps.append(
            {
                "hst": hst,
                "wq_t": np.ascontiguousarray(q_w[rs:re, :].T),
                "wk_t": np.ascontiguousarray(k_w[rs:re, :].T),
                "wv_t": np.ascontiguousarray(v_w[rs:re, :].T),
                "ow_t": np.ascontiguousarray(o_w[:, rs:re].T),
                "cos_t": cos_t,
                "sin_t": sin_t,
                "rperm": rperm,
                "ones": ones,
                "masks": masks,
            }
        )
    return in_maps


_NC_CACHE = {}


def _get_nc(T):
    if T not in _NC_CACHE:
        _NC_CACHE[T] = _build_nc(T)
    return _NC_CACHE[T]


def kernel(hidden_states, q_w, k_w, v_w, o_w, **run_kwargs):
    hidden_states = np.asarray(hidden_states, dtype=np.float32)
    q_w = np.asarray(q_w, dtype=np.float32)
    k_w = np.asarray(k_w, dtype=np.float32)
    v_w = np.asarray(v_w, dtype=np.float32)
    o_w = np.asarray(o_w, dtype=np.float32)
    T = hidden_states.shape[1]
    nc = _get_nc(T)
    in_maps = _host_prep(hidden_states, q_w, k_w, v_w, o_w)
    res = bass_utils.run_bass_kernel_spmd(
        nc, in_maps, core_ids=list(range(N_CORES)), **run_kwargs
    )
    out = np.zeros((B, T, C), dtype=np.float64)
    for r in res.results:
        out += r["out_p"].astype(np.float64)
    kernel.last_results = res
    return out.astype(np.float32)


# revision 22
# speedup vs baseline: 1.1492x; 1.1492x over previous
"""Trainium2 Bass kernel: causal multi-head attention with RoPE.

Reference computation (B=2, T=2048, C=2048, H=16, D=128, fp32):
    q/k/v = hs @ {q,k,v}_w^T ; RoPE(q), RoPE(k)
    out   = softmax(causal(q k^T / sqrt(D))) v @ o_w^T

Sharding: tensor-parallel over heads — each of the 8 cores owns 2 heads.
Each core computes its heads' projections + attention and a partial output
projection; the host sums the 8 partials.

Per-core device pipeline (all matmuls in float32r = full-rate fp32):
  A) stream hs^T chunks; qT/kT in [d, t] layout (per-window tiles so
     later stages only wait on the exact window they read), v in [t, d]
     layout; RoPE (rotate_half as a constant +-1 permutation matmul +
     cos/sin elementwise) interleaved per pair of chunks.
  B) scores computed TRANSPOSED [tk, tq]; exp on ACT with 1/sqrt(D)
     folded into the activation scale; causal masking as a 0/1 multiply
     on block-diagonal tiles only; softmax denominator via an M=1
     all-ones matmul accumulated alongside PV; unnormalized attnT
     overwrites the spent q window tiles.  Per head: reciprocal of the
     denominators via exp(-ln(x)) (one ACT table-set switch pair per
     head, overlapped with the next head's attention), gpsimd
     partition-broadcast, normalize in place.
  C) output projection interleaved with the last head's normalizes;
     partial [t, c] tiles to DRAM.
"""

import math
import sys

if "/opt/trn_rl_repo" not in sys.path:
    sys.path.insert(0, "/opt/trn_rl_repo")

import numpy as np

import concourse.bass as bass
import concourse.mybir as mybir
import concourse.tile as tile
from concourse import bacc, bass_utils

F32 = mybir.dt.float32
F32R = mybir.dt.float32r
AF = mybir.ActivationFunctionType
MULT = mybir.AluOpType.mult
ADD = mybir.AluOpType.add

B = 2
C = 2048
H = 16
D = 128
N_CORES = 8
HPC = H // N_CORES  # heads per core
DPC = HPC * D  # channels per core (256)
ROPE_BASE = 10000.0
P = 128  # partitions
TQW = 512  # tq window (matmul free dim)
TCH = 256  # hs^T chunk width in t


def _build_nc(T: int = 2048):
    """Build the per-core Bass program (SPMD: same program, per-core data)."""
    KT = C // P  # 16 k-tiles over the contraction dim c
    n_ch = T // TCH  # hs chunks per batch
    n_w = T // TQW  # tq windows per (b, h)
    cpw = TQW // TCH  # chunks per window
    scale = 1.0 / math.sqrt(D)

    nc = bacc.Bacc(trn_type="TRN2", target_bir_lowering=False, debug=False)

    hst = nc.dram_tensor("hst", [B, P, T // TCH, KT, TCH], F32R, kind="ExternalInput").ap()
    wq = nc.dram_tensor("wq_t", [P, KT, DPC], F32R, kind="ExternalInput").ap()
    wk = nc.dram_tensor("wk_t", [P, KT, DPC], F32R, kind="ExternalInput").ap()
    wv = nc.dram_tensor("wv_t", [P, KT, DPC], F32R, kind="ExternalInput").ap()
    ow = nc.dram_tensor("ow_t", [P, HPC, C], F32R, kind="ExternalInput").ap()
    cos_d = nc.dram_tensor("cos_t", [D, T], F32, kind="ExternalInput").ap()
    sin_d = nc.dram_tensor("sin_t", [D, T], F32, kind="ExternalInput").ap()
    rp_d = nc.dram_tensor("rperm", [D, D], F32R, kind="ExternalInput").ap()
    ones_d = nc.dram_tensor("ones", [P, 1], F32R, kind="ExternalInput").ap()
    msk_d = nc.dram_tensor("masks", [P, TQW // P, TQW], F32, kind="ExternalInput").ap()
    out_d = nc.dram_tensor("out_p", [B, T // P, C // TQW, P, TQW], F32, kind="ExternalOutput").ap()

    with tile.TileContext(nc) as tc:
        with (
            tc.tile_pool(name="consts", bufs=1) as consts,
            tc.tile_pool(name="hst", bufs=2) as hstp,
            tc.tile_pool(name="qkv", bufs=1) as qkvp,
            tc.tile_pool(name="exp", bufs=6) as expp,
            tc.tile_pool(name="small", bufs=2) as smallp,
            tc.tile_pool(name="outp", bufs=3) as outp,
            tc.tile_pool(name="ps", bufs=8, space="PSUM") as ps,
        ):
            # ---- resident constants -------------------------------------
            wq_sb = [
                consts.tile([P, 4, DPC], F32R, tag=f"wq{i}", name=f"wq{i}")
                for i in range(KT // 4)
            ]
            wk_sb = [
                consts.tile([P, 4, DPC], F32R, tag=f"wk{i}", name=f"wk{i}")
                for i in range(KT // 4)
            ]
            wv_sb = [
                consts.tile([P, 4, DPC], F32R, tag=f"wv{i}", name=f"wv{i}")
                for i in range(KT // 4)
            ]
            ow_sb = consts.tile([P, HPC, C], F32R, tag="ow")
            cos_sb = consts.tile([D, T], F32, tag="cos")
            sin_sb = consts.tile([D, T], F32, tag="sin")
            msk_sb = consts.tile([P, TQW // P, TQW], F32, tag="msk")
            ones_sb = consts.tile([P, 1], F32R, tag="ones")
            rp_sb = consts.tile([D, D], F32R, tag="rp")
            # Critical-path-first DMA order: the first chunk's matmuls
            # need only weight quarter 0 + hs chunk 0; everything else can
            # stream in behind them (queues drain FIFO).
            for w_sb, w_d in ((wq_sb, wq), (wk_sb, wk), (wv_sb, wv)):
                nc.sync.dma_start(w_sb[0][:], w_d[:, bass.ts(0, 4), :])
            pre_tiles = {}
            for ch in range(2):
                ht = hstp.tile([P, KT, TCH], F32R, tag="hst", name="ht_pre")
                nc.sync.dma_start(ht[:], hst[0, :, ch, :, :])
                pre_tiles[ch] = ht
            for i in (1, 2, 3):
                for w_sb, w_d in ((wq_sb, wq), (wk_sb, wk), (wv_sb, wv)):
                    nc.sync.dma_start(w_sb[i][:], w_d[:, bass.ts(i, 4), :])
            nc.sync.dma_start(cos_sb[:], cos_d)
            nc.sync.dma_start(sin_sb[:], sin_d)
            nc.sync.dma_start(rp_sb[:], rp_d)
            late_dmas_done = []

            for b in range(B):
                # Per-window q/k tiles: fine-grained deps (a window's
                # consumers only wait on that window's producers).
                q_t = [
                    [qkvp.tile([P, TQW], F32R, tag=f"q{h}w{w}", name=f"q{h}w{w}") for w in range(n_w)]
                    for h in range(HPC)
                ]
                k_t = [
                    [qkvp.tile([P, TQW], F32R, tag=f"k{h}w{w}", name=f"k{h}w{w}") for w in range(n_w)]
                    for h in range(HPC)
                ]
                v_sb = qkvp.tile([P, T // P, DPC], F32R, tag="v")
                # Broadcast denominator tiles: row 0 receives the den row,
                # gpsimd replicates it across partitions during attention.
                bc_t = [
                    [qkvp.tile([P, TQW], F32, tag=f"bc{h}w{w}", name=f"bc{h}w{w}") for w in range(n_w)]
                    for h in range(HPC)
                ]

                # ---- phase A: projections + RoPE ------------------------
                def rope(w, b=b):
                    sl = bass.ts(w, TQW)
                    for h in range(HPC):
                        for x_t in (q_t, k_t):
                            x = x_t[h][w]
                            rh = ps.tile([P, TQW], F32, tag="ps")
                            nc.tensor.matmul(
                                rh[:], rp_sb[:], x[:], start=True, stop=True
                            )
                            t1 = smallp.tile([P, TQW], F32, tag="t1")
                            nc.vector.tensor_tensor(
                                t1[:], x[:].bitcast(F32), cos_sb[:, sl], op=MULT
                            )
                            nc.vector.tensor_tensor(rh[:], rh[:], sin_sb[:, sl], op=MULT)
                            nc.vector.tensor_tensor(x[:], t1[:], rh[:], op=ADD)

                ctx_a = nc.named_scope(f"A{b}"); ctx_a.__enter__()
                for ch in range(n_ch):
                    w, cw = divmod(ch, cpw)
                    slc = bass.ts(cw, TCH)
                    if b == 0 and ch in pre_tiles:
                        hst_t = pre_tiles.pop(ch)
                    else:
                        hst_t = hstp.tile([P, KT, TCH], F32R, tag="hst")
                        nc.sync.dma_start(hst_t[:], hst[b, :, ch, :, :])
                    for h in range(HPC):
                        for x_t, w_sb in ((q_t, wq_sb), (k_t, wk_sb)):
                            pt = ps.tile([P, TQW], F32, tag="ps")
                            for k in range(KT):
                                nc.tensor.matmul(
                                    pt[:, :TCH],
                                    w_sb[k // 4][:, k % 4, bass.ts(h, D)],
                                    hst_t[:, k, :],
                                    start=(k == 0),
                                    stop=(k == KT - 1),
                                )
                            nc.scalar.activation(
                                x_t[h][w][:, slc], pt[:, :TCH], AF.Copy
                            )
                    for sub in range(TCH // P):
                        pt = ps.tile([P, TQW], F32, tag="ps")
                        for k in range(KT):
                            nc.tensor.matmul(
                                pt[:, :DPC],
                                hst_t[:, k, bass.ts(sub, P)],
                                wv_sb[k // 4][:, k % 4, :],
                                start=(k == 0),
                                stop=(k == KT - 1),
                            )
                        nc.scalar.activation(
                            v_sb[:, ch * (TCH // P) + sub, :], pt[:, :DPC], AF.Copy
                        )
                    # RoPE as soon as a full tq window of q/k is projected.
                    if cw == cpw - 1:
                        rope(w)

                if not late_dmas_done:
                    nc.sync.dma_start(msk_sb[:], msk_d)
                    nc.sync.dma_start(ones_sb[:], ones_d)
                ctx_a.__exit__(None, None, None)
                # ---- phase B: attention (unnormalized) ------------------
                def attend(h):
                    for w in range(n_w):
                        ntk = (w + 1) * (TQW // P)

                        def qk_exp(i, h=h, w=w):
                            st = ps.tile([P, TQW], F32, tag="ps")
                            nc.tensor.matmul(
                                st[:],
                                k_t[h][i // (TQW // P)][:, bass.ts(i % (TQW // P), P)],
                                q_t[h][w][:],
                                start=True,
                                stop=True,
                            )
                            e = expp.tile([P, TQW], F32R, tag="exp")
                            nc.scalar.activation(e[:], st[:], AF.Exp, scale=scale)
                            off = i - w * (TQW // P)
                            if off >= 0:
                                nc.vector.tensor_tensor(
                                    e[:], e[:].bitcast(F32), msk_sb[:, off, :], op=MULT
                                )
                            return e

                        fifo = [qk_exp(j) for j in range(min(3, ntk))]
                        pv = ps.tile([P, TQW], F32, tag="ps")
                        den = ps.tile([P, TQW], F32, tag="ps")
                        for i in range(ntk):
                            if i + 3 < ntk:
                                fifo.append(qk_exp(i + 3))
                            e = fifo.pop(0)
                            nc.tensor.matmul(
                                pv[:],
                                v_sb[:, i, bass.ts(h, D)],
                                e[:],
                                start=(i == 0),
                                stop=(i == ntk - 1),
                            )
                            nc.tensor.matmul(
                                den[:1, :],
                                ones_sb[:],
                                e[:],
                                start=(i == 0),
                                stop=(i == ntk - 1),
                            )
                        # stash unnormalized attnT into the spent q window;
                        # den row -> bc tile row 0, replicate on gpsimd, then
                        # 1/x on DVE (2 custom ops, ~2 ULP, no ACT tables)
                        # and normalize the window in place.
                        nc.vector.tensor_copy(q_t[h][w][:], pv[:])
                        bc = bc_t[h][w]
                        nc.vector.tensor_copy(bc[:1, :], den[:1, :])
                        nc.gpsimd.partition_broadcast(bc[:], bc[:1, :])
                        scr = smallp.tile([P, TQW], F32, tag="t1", name="scr")
                        nc.vector.reciprocal_approx_accurate(
                            out=bc[:], in_=bc[:], scratch=scr[:]
                        )
                        nc.vector.tensor_tensor(
                            q_t[h][w][:], q_t[h][w][:].bitcast(F32), bc[:], op=MULT
                        )

                with nc.named_scope(f"B{b}"):
                    attend(0)
                    attend(1)

                # ---- phase C: output projection, interleaved with the
                # normalizes ----------------------------------------------
                if not late_dmas_done:
                    nc.sync.dma_start(ow_sb[:], ow)
                    late_dmas_done.append(True)
                ctx_c = nc.named_scope(f"C{b}"); ctx_c.__enter__()
                for w in range(n_w):
                    for m in range(w * (TQW // P), (w + 1) * (TQW // P)):
                        for n in range(C // TQW):
                            po = ps.tile([P, TQW], F32, tag="ps")
                            for h in range(HPC):
                                nc.tensor.matmul(
                                    po[:],
                                    q_t[h][m // (TQW // P)][
                                        :, bass.ts(m % (TQW // P), P)
                                    ],
                                    ow_sb[:, h, bass.ts(n, TQW)],
                                    start=(h == 0),
                                    stop=(h == HPC - 1),
                                )
                            o_t = outp.tile([P, TQW], F32, tag="o")
                            nc.any.tensor_copy(o_t[:], po[:])
                            nc.sync.dma_start(out_d[b, m, n], o_t[:])
                ctx_c.__exit__(None, None, None)

    nc.compile()
    return nc


def _host_prep(hidden_states, q_w, k_w, v_w, o_w):
    """Build the 8 per-core input maps (and shared constant tensors)."""
    T = hidden_states.shape[1]
    f32 = np.float32

    n_ch = T // TCH
    KT = C // P
    # [B, T, C] -> hs^T blocked per (partition, chunk): [B, P, n_ch, KT, TCH]
    hstT = hidden_states.transpose(0, 2, 1)  # [B, C, T]
    hst = np.ascontiguousarray(
        hstT.reshape(B, KT, P, n_ch, TCH).transpose(0, 2, 3, 1, 4)
    ).astype(f32, copy=False)

    def wblk(w_slice):
        # [DPC, C] row-slice -> w^T blocked [P, KT, DPC]
        return np.ascontiguousarray(
            w_slice.T.reshape(KT, P, DPC).transpose(1, 0, 2)
        ).astype(f32, copy=False)


    inv_freq = 1.0 / (ROPE_BASE ** (np.arange(0, D, 2, dtype=np.float64) / D))
    t_ar = np.arange(T, dtype=np.float64)
    freqs = t_ar[:, None] * inv_freq[None, :]  # [T, D/2]
    cos_td = np.concatenate([np.cos(freqs), np.cos(freqs)], axis=-1)  # [T, D]
    sin_td = np.concatenate([np.sin(freqs), np.sin(freqs)], axis=-1)
    cos_t = np.ascontiguousarray(cos_td.T).astype(f32)  # [D, T]
    sin_t = np.ascontiguousarray(sin_td.T).astype(f32)

    # rotate_half as a matmul: rh = R @ x ; rperm = R^T (lhsT operand).
    rperm = np.zeros((D, D), dtype=f32)
    half = D // 2
    for j in range(half):
        rperm[2 * j + 1, j] = -1.0
    for j in range(half, D):
        rperm[2 * (j - half), j] = 1.0

    ones = np.ones((P, 1), dtype=f32)

    n_off = TQW // P
    masks = np.zeros((P, n_off, TQW), dtype=f32)
    y = np.arange(P)[:, None]
    x = np.arange(TQW)[None, :]
    for o in range(n_off):
        masks[:, o, :] = (x >= P * o + y).astype(f32)

    in_maps = []
    for c in range(N_CORES):
        rs, re = c * DPC, (c + 1) * DPC
        in_maps.append(
            {
                "hst": hst,
                "wq_t": wblk(q_w[rs:re, :]),
                "wk_t": wblk(k_w[rs:re, :]),
                "wv_t": wblk(v_w[rs:re, :]),
                "ow_t": np.ascontiguousarray(o_w[:, rs:re].T.reshape(HPC, P, C).transpose(1, 0, 2)),
                "cos_t": cos_t,
                "sin_t": sin_t,
                "rperm": rperm,
                "ones": ones,
                "masks": masks,
            }
        )
    return in_maps


_NC_CACHE = {}


def _get_nc(T):
    if T not in _NC_CACHE:
        _NC_CACHE[T] = _build_nc(T)
    return _NC_CACHE[T]


def kernel(hidden_states, q_w, k_w, v_w, o_w, **run_kwargs):
    hidden_states = np.asarray(hidden_states, dtype=np.float32)
    q_w = np.asarray(q_w, dtype=np.float32)
    k_w = np.asarray(k_w, dtype=np.float32)
    v_w = np.asarray(v_w, dtype=np.float32)
    o_w = np.asarray(o_w, dtype=np.float32)
    T = hidden_states.shape[1]
    nc = _get_nc(T)
    in_maps = _host_prep(hidden_states, q_w, k_w, v_w, o_w)
    res = bass_utils.run_bass_kernel_spmd(
        nc, in_maps, core_ids=list(range(N_CORES)), **run_kwargs
    )
    out = np.zeros((B, T // P, C // TQW, P, TQW), dtype=np.float64)
    for r in res.results:
        out += r["out_p"].astype(np.float64)
    kernel.last_results = res
    return (
        out.transpose(0, 1, 3, 2, 4).reshape(B, T, C).astype(np.float32)
    )


# revision 27
# speedup vs baseline: 1.2167x; 1.0588x over previous
"""Trainium2 Bass kernel: causal multi-head attention with RoPE.

Reference computation (B=2, T=2048, C=2048, H=16, D=128, fp32):
    q/k/v = hs @ {q,k,v}_w^T ; RoPE(q), RoPE(k)
    out   = softmax(causal(q k^T / sqrt(D))) v @ o_w^T

Sharding: tensor-parallel over heads — each of the 8 cores owns 2 heads.
Each core computes its heads' projections + attention and a partial output
projection; the host sums the 8 partials.

Per-core device pipeline (all matmuls in float32r = full-rate fp32):
  A) stream hs^T chunks; qT/kT in [d, t] layout (per-window tiles so
     later stages only wait on the exact window they read), v in [t, d]
     layout; RoPE (rotate_half as a constant +-1 permutation matmul +
     cos/sin elementwise) interleaved per pair of chunks.
  B) scores computed TRANSPOSED [tk, tq]; exp on ACT with 1/sqrt(D)
     folded into the activation scale; causal masking as a 0/1 multiply
     on block-diagonal tiles only; softmax denominator via an M=1
     all-ones matmul accumulated alongside PV; unnormalized attnT
     overwrites the spent q window tiles.  Per head: reciprocal of the
     denominators via exp(-ln(x)) (one ACT table-set switch pair per
     head, overlapped with the next head's attention), gpsimd
     partition-broadcast, normalize in place.
  C) output projection interleaved with the last head's normalizes;
     partial [t, c] tiles to DRAM.
"""

import math
import sys

if "/opt/trn_rl_repo" not in sys.path:
    sys.path.insert(0, "/opt/trn_rl_repo")

import numpy as np

import concourse.bass as bass
import concourse.mybir as mybir
import concourse.tile as tile
from concourse import bacc, bass_utils

F32 = mybir.dt.float32
F32R = mybir.dt.float32r
AF = mybir.ActivationFunctionType
MULT = mybir.AluOpType.mult
ADD = mybir.AluOpType.add

B = 2
C = 2048
H = 16
D = 128
N_CORES = 8
HPC = H // N_CORES  # heads per core
DPC = HPC * D  # channels per core (256)
ROPE_BASE = 10000.0
P = 128  # partitions
TQW = 512  # tq window (matmul free dim)
TCH = 256  # hs^T chunk width in t


def _build_nc(T: int = 2048):
    """Build the per-core Bass program (SPMD: same program, per-core data)."""
    KT = C // P  # 16 k-tiles over the contraction dim c
    n_ch = T // TCH  # hs chunks per batch
    n_w = T // TQW  # tq windows per (b, h)
    cpw = TQW // TCH  # chunks per window
    scale = 1.0 / math.sqrt(D)

    nc = bacc.Bacc(trn_type="TRN2", target_bir_lowering=False, debug=False)

    hst = nc.dram_tensor("hst", [B, P, T // TQW, KT // 4, 4, TQW], F32R, kind="ExternalInput").ap()
    wq = nc.dram_tensor("wq_t", [P, KT, DPC], F32R, kind="ExternalInput").ap()
    wk = nc.dram_tensor("wk_t", [P, KT, DPC], F32R, kind="ExternalInput").ap()
    wv = nc.dram_tensor("wv_t", [P, KT, DPC], F32R, kind="ExternalInput").ap()
    ow = nc.dram_tensor("ow_t", [P, HPC, C], F32R, kind="ExternalInput").ap()
    cos_d = nc.dram_tensor("cos_t", [D, T], F32, kind="ExternalInput").ap()
    sin_d = nc.dram_tensor("sin_t", [D, T], F32, kind="ExternalInput").ap()
    rp_d = nc.dram_tensor("rperm", [D, D], F32R, kind="ExternalInput").ap()
    ones_d = nc.dram_tensor("ones", [P, 1], F32R, kind="ExternalInput").ap()
    msk_d = nc.dram_tensor("masks", [P, TQW // P, TQW], F32, kind="ExternalInput").ap()
    out_d = nc.dram_tensor("out_p", [B, T // P, C // TQW, P, TQW], F32, kind="ExternalOutput").ap()

    with tile.TileContext(nc) as tc:
        with (
            tc.tile_pool(name="consts", bufs=1) as consts,
            tc.tile_pool(name="hst", bufs=4) as hstp,
            tc.tile_pool(name="qkv", bufs=1) as qkvp,
            tc.tile_pool(name="exp", bufs=6) as expp,
            tc.tile_pool(name="small", bufs=2) as smallp,
            tc.tile_pool(name="outp", bufs=3) as outp,
            tc.tile_pool(name="ps", bufs=8, space="PSUM") as ps,
        ):
            # ---- resident constants -------------------------------------
            wq_sb = [
                consts.tile([P, 4, DPC], F32R, tag=f"wq{i}", name=f"wq{i}")
                for i in range(KT // 4)
            ]
            wk_sb = [
                consts.tile([P, 4, DPC], F32R, tag=f"wk{i}", name=f"wk{i}")
                for i in range(KT // 4)
            ]
            wv_sb = [
                consts.tile([P, 4, DPC], F32R, tag=f"wv{i}", name=f"wv{i}")
                for i in range(KT // 4)
            ]
            ow_sb = consts.tile([P, HPC, C], F32R, tag="ow")
            cos_sb = consts.tile([D, T], F32, tag="cos")
            sin_sb = consts.tile([D, T], F32, tag="sin")
            msk_sb = consts.tile([P, TQW // P, TQW], F32, tag="msk")
            ones_sb = consts.tile([P, 1], F32R, tag="ones")
            rp_sb = consts.tile([D, D], F32R, tag="rp")
            # Critical-path-first DMA order: the first chunk's matmuls
            # need only weight quarter 0 + hs chunk 0; everything else can
            # stream in behind them (queues drain FIFO).
            for w_sb, w_d in ((wq_sb, wq), (wk_sb, wk), (wv_sb, wv)):
                nc.sync.dma_start(w_sb[0][:], w_d[:, bass.ts(0, 4), :])
            pre_tiles = {}
            for qi in range(4):
                ht = hstp.tile([P, 4, TQW], F32R, tag="hst", name="ht_pre")
                nc.sync.dma_start(ht[:], hst[0, :, 0, qi, :, :])
                pre_tiles[qi] = ht
            for i in (1, 2, 3):
                for w_sb, w_d in ((wq_sb, wq), (wk_sb, wk), (wv_sb, wv)):
                    nc.sync.dma_start(w_sb[i][:], w_d[:, bass.ts(i, 4), :])
            nc.sync.dma_start(cos_sb[:], cos_d)
            nc.sync.dma_start(sin_sb[:], sin_d)
            nc.sync.dma_start(rp_sb[:], rp_d)
            late_dmas_done = []

            for b in range(B):
                # Per-window q/k tiles: fine-grained deps (a window's
                # consumers only wait on that window's producers).
                q_t = [
                    [qkvp.tile([P, TQW], F32R, tag=f"q{h}w{w}", name=f"q{h}w{w}") for w in range(n_w)]
                    for h in range(HPC)
                ]
                k_t = [
                    [qkvp.tile([P, TQW], F32R, tag=f"k{h}w{w}", name=f"k{h}w{w}") for w in range(n_w)]
                    for h in range(HPC)
                ]
                v_sb = qkvp.tile([P, T // P, DPC], F32R, tag="v")
                # Broadcast denominator tiles: row 0 receives the den row,
                # gpsimd replicates it across partitions during attention.
                bc_t = [
                    [qkvp.tile([P, TQW], F32, tag=f"bc{h}w{w}", name=f"bc{h}w{w}") for w in range(n_w)]
                    for h in range(HPC)
                ]

                # ---- phase A: projections + RoPE ------------------------
                def rope(w, b=b):
                    sl = bass.ts(w, TQW)
                    for h in range(HPC):
                        for x_t in (q_t, k_t):
                            x = x_t[h][w]
                            rh = ps.tile([P, TQW], F32, tag="ps")
                            nc.tensor.matmul(
                                rh[:], rp_sb[:], x[:], start=True, stop=True
                            )
                            t1 = smallp.tile([P, TQW], F32, tag="t1")
                            nc.vector.tensor_tensor(
                                t1[:], x[:].bitcast(F32), cos_sb[:, sl], op=MULT
                            )
                            nc.vector.tensor_tensor(rh[:], rh[:], sin_sb[:, sl], op=MULT)
                            nc.vector.tensor_tensor(x[:], t1[:], rh[:], op=ADD)

                ctx_a = nc.named_scope(f"A{b}"); ctx_a.__enter__()
                for w in range(n_w):
                    hts = []
                    for qi in range(4):
                        if b == 0 and w == 0 and qi in pre_tiles:
                            ht = pre_tiles.pop(qi)
                        else:
                            ht = hstp.tile([P, 4, TQW], F32R, tag="hst", name="ht")
                            nc.sync.dma_start(ht[:], hst[b, :, w, qi, :, :])
                        hts.append(ht)
                    pq = [ps.tile([P, TQW], F32, tag="ps", name="pq") for _ in range(HPC)]
                    pk = [ps.tile([P, TQW], F32, tag="ps", name="pk") for _ in range(HPC)]
                    for k in range(KT):
                        for h in range(HPC):
                            for pt, w_sb in ((pq[h], wq_sb), (pk[h], wk_sb)):
                                nc.tensor.matmul(
                                    pt[:],
                                    w_sb[k // 4][:, k % 4, bass.ts(h, D)],
                                    hts[k // 4][:, k % 4, :],
                                    start=(k == 0),
                                    stop=(k == KT - 1),
                                )
                    # Rank the psum->sbuf copies later so attention's first
                    # exps win the ACT queue at the phase A->B transition
                    # (deps still force early-window copies on time).
                    with tc.high_priority(-600):
                        for h in range(HPC):
                            nc.scalar.activation(q_t[h][w][:], pq[h][:], AF.Copy)
                            nc.scalar.activation(k_t[h][w][:], pk[h][:], AF.Copy)
                    pv4 = [
                        ps.tile([P, DPC], F32, tag="ps", name="pv4")
                        for _ in range(TQW // P)
                    ]
                    for k in range(KT):
                        for sub in range(TQW // P):
                            nc.tensor.matmul(
                                pv4[sub][:],
                                hts[k // 4][:, k % 4, bass.ts(sub, P)],
                                wv_sb[k // 4][:, k % 4, :],
                                start=(k == 0),
                                stop=(k == KT - 1),
                            )
                    with tc.high_priority(-600):
                        for sub in range(TQW // P):
                            nc.scalar.activation(
                                v_sb[:, w * (TQW // P) + sub, :], pv4[sub][:], AF.Copy
                            )
                    rope(w)
                if not late_dmas_done:
                    nc.sync.dma_start(msk_sb[:], msk_d)
                    nc.sync.dma_start(ones_sb[:], ones_d)
                ctx_a.__exit__(None, None, None)
                # ---- phase B: attention (unnormalized) ------------------
                def attend_win(h, w):
                    if True:
                        ntk = (w + 1) * (TQW // P)

                        def qk_exp(i, h=h, w=w):
                            st = ps.tile([P, TQW], F32, tag="ps")
                            nc.tensor.matmul(
                                st[:],
                                k_t[h][i // (TQW // P)][:, bass.ts(i % (TQW // P), P)],
                                q_t[h][w][:],
                                start=True,
                                stop=True,
                            )
                            e = expp.tile([P, TQW], F32R, tag="exp")
                            nc.scalar.activation(e[:], st[:], AF.Exp, scale=scale)
                            off = i - w * (TQW // P)
                            if off >= 0:
                                nc.vector.tensor_tensor(
                                    e[:], e[:].bitcast(F32), msk_sb[:, off, :], op=MULT
                                )
                            return e

                        fifo = [qk_exp(j) for j in range(min(3, ntk))]
                        pv = ps.tile([P, TQW], F32, tag="ps")
                        den = ps.tile([P, TQW], F32, tag="ps")
                        for i in range(ntk):
                            if i + 3 < ntk:
                                fifo.append(qk_exp(i + 3))
                            e = fifo.pop(0)
                            nc.tensor.matmul(
                                pv[:],
                                v_sb[:, i, bass.ts(h, D)],
                                e[:],
                                start=(i == 0),
                                stop=(i == ntk - 1),
                            )
                            nc.tensor.matmul(
                                den[:1, :],
                                ones_sb[:],
                                e[:],
                                start=(i == 0),
                                stop=(i == ntk - 1),
                            )
                        # stash unnormalized attnT into the spent q window;
                        # den row -> bc tile row 0, replicate on gpsimd, then
                        # 1/x on DVE (2 custom ops, ~2 ULP, no ACT tables)
                        # and normalize the window in place.
                        nc.vector.tensor_copy(q_t[h][w][:], pv[:])
                        bc = bc_t[h][w]
                        nc.vector.tensor_copy(bc[:1, :], den[:1, :])
                        nc.gpsimd.partition_broadcast(bc[:], bc[:1, :])
                        scr = smallp.tile([P, TQW], F32, tag="t1", name="scr")
                        nc.vector.reciprocal_approx_accurate(
                            out=bc[:], in_=bc[:], scratch=scr[:]
                        )
                        nc.vector.tensor_tensor(
                            q_t[h][w][:], q_t[h][w][:].bitcast(F32), bc[:], op=MULT
                        )

                def phase_c_win(w):
                    for m in range(w * (TQW // P), (w + 1) * (TQW // P)):
                        for n in range(C // TQW):
                            po = ps.tile([P, TQW], F32, tag="ps")
                            for h in range(HPC):
                                nc.tensor.matmul(
                                    po[:],
                                    q_t[h][m // (TQW // P)][
                                        :, bass.ts(m % (TQW // P), P)
                                    ],
                                    ow_sb[:, h, bass.ts(n, TQW)],
                                    start=(h == 0),
                                    stop=(h == HPC - 1),
                                )
                            o_t = outp.tile([P, TQW], F32, tag="o")
                            nc.any.tensor_copy(o_t[:], po[:])
                            nc.sync.dma_start(out_d[b, m, n], o_t[:])

                # ---- attention + output projection, software-pipelined:
                # phase C of window w-1 runs between attention windows so
                # the output DMA overlaps compute (only the last window's
                # store is tail-exposed).
                if not late_dmas_done:
                    nc.sync.dma_start(ow_sb[:], ow)
                    late_dmas_done.append(True)
                with nc.named_scope(f"BC{b}"):
                    for w in range(n_w):
                        attend_win(0, w)
                        attend_win(1, w)
                        if w > 0:
                            phase_c_win(w - 1)
                    phase_c_win(n_w - 1)

    nc.compile()
    return nc


def _host_prep(hidden_states, q_w, k_w, v_w, o_w):
    """Build the 8 per-core input maps (and shared constant tensors)."""
    T = hidden_states.shape[1]
    f32 = np.float32

    n_w = T // TQW
    KT = C // P
    # [B, T, C] -> hs^T blocked per (partition, window, k-quarter):
    # [B, P, n_w, KT//4, 4, TQW]
    hstT = hidden_states.transpose(0, 2, 1)  # [B, C, T]
    hst = np.ascontiguousarray(
        hstT.reshape(B, KT // 4, 4, P, n_w, TQW).transpose(0, 3, 4, 1, 2, 5)
    ).astype(f32, copy=False)

    def wblk(w_slice):
        # [DPC, C] row-slice -> w^T blocked [P, KT, DPC]
        return np.ascontiguousarray(
            w_slice.T.reshape(KT, P, DPC).transpose(1, 0, 2)
        ).astype(f32, copy=False)


    inv_freq = 1.0 / (ROPE_BASE ** (np.arange(0, D, 2, dtype=np.float64) / D))
    t_ar = np.arange(T, dtype=np.float64)
    freqs = t_ar[:, None] * inv_freq[None, :]  # [T, D/2]
    cos_td = np.concatenate([np.cos(freqs), np.cos(freqs)], axis=-1)  # [T, D]
    sin_td = np.concatenate([np.sin(freqs), np.sin(freqs)], axis=-1)
    cos_t = np.ascontiguousarray(cos_td.T).astype(f32)  # [D, T]
    sin_t = np.ascontiguousarray(sin_td.T).astype(f32)

    # rotate_half as a matmul: rh = R @ x ; rperm = R^T (lhsT operand).
    rperm = np.zeros((D, D), dtype=f32)
    half = D // 2
    for j in range(half):
        rperm[2 * j + 1, j] = -1.0
    for j in range(half, D):
        rperm[2 * (j - half), j] = 1.0

    ones = np.ones((P, 1), dtype=f32)

    n_off = TQW // P
    masks = np.zeros((P, n_off, TQW), dtype=f32)
    y = np.arange(P)[:, None]
    x = np.arange(TQW)[None, :]
    for o in range(n_off):
        masks[:, o, :] = (x >= P * o + y).astype(f32)

    in_maps = []
    for c in range(N_CORES):
        rs, re = c * DPC, (c + 1) * DPC
        in_maps.append(
            {
                "hst": hst,
                "wq_t": wblk(q_w[rs:re, :]),
                "wk_t": wblk(k_w[rs:re, :]),
                "wv_t": wblk(v_w[rs:re, :]),
                "ow_t": np.ascontiguousarray(o_w[:, rs:re].T.reshape(HPC, P, C).transpose(1, 0, 2)),
                "cos_t": cos_t,
                "sin_t": sin_t,
                "rperm": rperm,
                "ones": ones,
                "masks": masks,
            }
        )
    return in_maps


_NC_CACHE = {}


def _get_nc(T):
    if T not in _NC_CACHE:
        _NC_CACHE[T] = _build_nc(T)
    return _NC_CACHE[T]


def kernel(hidden_states, q_w, k_w, v_w, o_w, **run_kwargs):
    hidden_states = np.asarray(hidden_states, dtype=np.float32)
    q_w = np.asarray(q_w, dtype=np.float32)
    k_w = np.asarray(k_w, dtype=np.float32)
    v_w = np.asarray(v_w, dtype=np.float32)
    o_w = np.asarray(o_w, dtype=np.float32)
    T = hidden_states.shape[1]
    nc = _get_nc(T)
    in_maps = _host_prep(hidden_states, q_w, k_w, v_w, o_w)
    res = bass_utils.run_bass_kernel_spmd(
        nc, in_maps, core_ids=list(range(N_CORES)), **run_kwargs
    )
    out = np.zeros((B, T // P, C // TQW, P, TQW), dtype=np.float64)
    for r in res.results:
        out += r["out_p"].astype(np.float64)
    kernel.last_results = res
    return (
        out.transpose(0, 1, 3, 2, 4).reshape(B, T, C).astype(np.float32)
    )


# revision 31
# speedup vs baseline: 1.3005x; 1.0689x over previous
"""Trainium2 Bass kernel: causal multi-head attention with RoPE.

Reference computation (B=2, T=2048, C=2048, H=16, D=128, fp32):
    q/k/v = hs @ {q,k,v}_w^T ; RoPE(q), RoPE(k)
    out   = softmax(causal(q k^T / sqrt(D))) v @ o_w^T

Sharding: tensor-parallel over heads — each of the 8 cores owns 2 heads.
Each core computes its heads' projections + attention and a partial output
projection; the host sums the 8 partials.

Per-core device pipeline (all matmuls in float32r = full-rate fp32):
  A) stream hs^T chunks; qT/kT in [d, t] layout (per-window tiles so
     later stages only wait on the exact window they read), v in [t, d]
     layout; RoPE (rotate_half as a constant +-1 permutation matmul +
     cos/sin elementwise) interleaved per pair of chunks.
  B) scores computed TRANSPOSED [tk, tq]; exp on ACT with 1/sqrt(D)
     folded into the activation scale; causal masking as a 0/1 multiply
     on block-diagonal tiles only; softmax denominator via an M=1
     all-ones matmul accumulated alongside PV; unnormalized attnT
     overwrites the spent q window tiles.  Per head: reciprocal of the
     denominators via exp(-ln(x)) (one ACT table-set switch pair per
     head, overlapped with the next head's attention), gpsimd
     partition-broadcast, normalize in place.
  C) output projection interleaved with the last head's normalizes;
     partial [t, c] tiles to DRAM.
"""

import math
import sys

if "/opt/trn_rl_repo" not in sys.path:
    sys.path.insert(0, "/opt/trn_rl_repo")

import numpy as np

import concourse.bass as bass
import concourse.mybir as mybir
import concourse.tile as tile
from concourse import bacc, bass_utils

F32 = mybir.dt.float32
F32R = mybir.dt.float32r
AF = mybir.ActivationFunctionType
MULT = mybir.AluOpType.mult
ADD = mybir.AluOpType.add

B = 2
C = 2048
H = 16
D = 128
N_CORES = 8
HPC = H // N_CORES  # heads per core
DPC = HPC * D  # channels per core (256)
ROPE_BASE = 10000.0
P = 128  # partitions
TQW = 512  # tq window (matmul free dim)
TCH = 256  # hs^T chunk width in t


def _build_nc(T: int = 2048):
    """Build the per-core Bass program (SPMD: same program, per-core data)."""
    KT = C // P  # 16 k-tiles over the contraction dim c
    n_ch = T // TCH  # hs chunks per batch
    n_w = T // TQW  # tq windows per (b, h)
    cpw = TQW // TCH  # chunks per window
    scale = 1.0 / math.sqrt(D)

    nc = bacc.Bacc(trn_type="TRN2", target_bir_lowering=False, debug=False)

    hst = nc.dram_tensor("hst", [B, P, T // TQW, KT // 4, 4, TQW], F32R, kind="ExternalInput").ap()
    wq = nc.dram_tensor("wq_t", [P, KT, DPC], F32R, kind="ExternalInput").ap()
    wk = nc.dram_tensor("wk_t", [P, KT, DPC], F32R, kind="ExternalInput").ap()
    wv = nc.dram_tensor("wv_t", [P, KT, DPC], F32R, kind="ExternalInput").ap()
    ow = nc.dram_tensor("ow_t", [P, HPC, C], F32R, kind="ExternalInput").ap()
    cos_d = nc.dram_tensor("cos_t", [D, T], F32, kind="ExternalInput").ap()
    sin_d = nc.dram_tensor("sin_t", [D, T], F32, kind="ExternalInput").ap()
    rp_d = nc.dram_tensor("rperm", [D, D], F32R, kind="ExternalInput").ap()
    ones_d = nc.dram_tensor("ones", [P, 1], F32R, kind="ExternalInput").ap()
    msk_d = nc.dram_tensor("masks", [P, TQW // P, TQW], F32, kind="ExternalInput").ap()
    out_d = nc.dram_tensor("out_p", [B, T // P, C // TQW, P, TQW], F32, kind="ExternalOutput").ap()

    with tile.TileContext(nc) as tc:
        with (
            tc.tile_pool(name="consts", bufs=1) as consts,
            tc.tile_pool(name="hst", bufs=4) as hstp,
            tc.tile_pool(name="qkv", bufs=1) as qkvp,
            tc.tile_pool(name="exp", bufs=6) as expp,
            tc.tile_pool(name="small", bufs=2) as smallp,
            tc.tile_pool(name="outp", bufs=3) as outp,
            tc.tile_pool(name="ps", bufs=8, space="PSUM") as ps,
        ):
            # ---- resident constants -------------------------------------
            wq_sb = [
                consts.tile([P, 4, DPC], F32R, tag=f"wq{i}", name=f"wq{i}")
                for i in range(KT // 4)
            ]
            wk_sb = [
                consts.tile([P, 4, DPC], F32R, tag=f"wk{i}", name=f"wk{i}")
                for i in range(KT // 4)
            ]
            wv_sb = [
                consts.tile([P, 4, DPC], F32R, tag=f"wv{i}", name=f"wv{i}")
                for i in range(KT // 4)
            ]
            ow_sb = consts.tile([P, HPC, C], F32R, tag="ow")
            cos_sb = consts.tile([D, T], F32, tag="cos")
            sin_sb = consts.tile([D, T], F32, tag="sin")
            msk_sb = consts.tile([P, TQW // P, TQW], F32, tag="msk")
            ones_sb = consts.tile([P, 1], F32R, tag="ones")
            rp_sb = consts.tile([D, D], F32R, tag="rp")
            # Critical-path-first DMA order: the first chunk's matmuls
            # need only weight quarter 0 + hs chunk 0; everything else can
            # stream in behind them (queues drain FIFO).
            for w_sb, w_d in ((wq_sb, wq), (wk_sb, wk), (wv_sb, wv)):
                nc.sync.dma_start(w_sb[0][:], w_d[:, bass.ts(0, 4), :])
            pre_tiles = {}
            for qi in range(4):
                ht = hstp.tile([P, 4, TQW], F32R, tag="hst", name="ht_pre")
                nc.sync.dma_start(ht[:], hst[0, :, 0, qi, :, :])
                pre_tiles[qi] = ht
            for i in (1, 2, 3):
                for w_sb, w_d in ((wq_sb, wq), (wk_sb, wk), (wv_sb, wv)):
                    nc.sync.dma_start(w_sb[i][:], w_d[:, bass.ts(i, 4), :])
            nc.sync.dma_start(cos_sb[:], cos_d)
            nc.sync.dma_start(sin_sb[:], sin_d)
            nc.sync.dma_start(rp_sb[:], rp_d)
            late_dmas_done = []

            for b in range(B):
                # Per-window q/k tiles: fine-grained deps (a window's
                # consumers only wait on that window's producers).
                q_t = [
                    [qkvp.tile([P, TQW], F32R, tag=f"q{h}w{w}", name=f"q{h}w{w}") for w in range(n_w)]
                    for h in range(HPC)
                ]
                k_t = [
                    [qkvp.tile([P, TQW], F32R, tag=f"k{h}w{w}", name=f"k{h}w{w}") for w in range(n_w)]
                    for h in range(HPC)
                ]
                v_sb = qkvp.tile([P, T // P, DPC], F32R, tag="v")
                # Broadcast denominator tiles: row 0 receives the den row,
                # gpsimd replicates it across partitions during attention.
                bc_t = [
                    [qkvp.tile([P, TQW], F32, tag=f"bc{h}w{w}", name=f"bc{h}w{w}") for w in range(n_w)]
                    for h in range(HPC)
                ]

                # ---- phase A: projections + RoPE ------------------------
                def rope(w, b=b):
                    sl = bass.ts(w, TQW)
                    for h in range(HPC):
                        for x_t in (q_t, k_t):
                            x = x_t[h][w]
                            rh = ps.tile([P, TQW], F32, tag="ps")
                            nc.tensor.matmul(
                                rh[:], rp_sb[:], x[:], start=True, stop=True
                            )
                            t1 = smallp.tile([P, TQW], F32, tag="t1")
                            nc.vector.tensor_tensor(
                                t1[:], x[:].bitcast(F32), cos_sb[:, sl], op=MULT
                            )
                            nc.vector.tensor_tensor(rh[:], rh[:], sin_sb[:, sl], op=MULT)
                            nc.vector.tensor_tensor(x[:], t1[:], rh[:], op=ADD)

                ctx_a = nc.named_scope(f"A{b}"); ctx_a.__enter__()
                for w in range(n_w):
                    hts = []
                    for qi in range(4):
                        if b == 0 and w == 0 and qi in pre_tiles:
                            ht = pre_tiles.pop(qi)
                        else:
                            ht = hstp.tile([P, 4, TQW], F32R, tag="hst", name="ht")
                            nc.sync.dma_start(ht[:], hst[b, :, w, qi, :, :])
                        hts.append(ht)
                    pq = [ps.tile([P, TQW], F32, tag="ps", name="pq") for _ in range(HPC)]
                    pk = [ps.tile([P, TQW], F32, tag="ps", name="pk") for _ in range(HPC)]
                    for k in range(KT):
                        for h in range(HPC):
                            for pt, w_sb in ((pq[h], wq_sb), (pk[h], wk_sb)):
                                nc.tensor.matmul(
                                    pt[:],
                                    w_sb[k // 4][:, k % 4, bass.ts(h, D)],
                                    hts[k // 4][:, k % 4, :],
                                    start=(k == 0),
                                    stop=(k == KT - 1),
                                )
                    # Rank the psum->sbuf copies later so attention's first
                    # exps win the ACT queue at the phase A->B transition
                    # (deps still force early-window copies on time).
                    with tc.high_priority(-600):
                        for h in range(HPC):
                            nc.scalar.activation(q_t[h][w][:], pq[h][:], AF.Copy)
                            nc.scalar.activation(k_t[h][w][:], pk[h][:], AF.Copy)
                    pv4 = [
                        ps.tile([P, DPC], F32, tag="ps", name="pv4")
                        for _ in range(TQW // P)
                    ]
                    for k in range(KT):
                        for sub in range(TQW // P):
                            nc.tensor.matmul(
                                pv4[sub][:],
                                hts[k // 4][:, k % 4, bass.ts(sub, P)],
                                wv_sb[k // 4][:, k % 4, :],
                                start=(k == 0),
                                stop=(k == KT - 1),
                            )
                    with tc.high_priority(-600):
                        for sub in range(TQW // P):
                            nc.scalar.activation(
                                v_sb[:, w * (TQW // P) + sub, :], pv4[sub][:], AF.Copy
                            )
                    rope(w)
                if not late_dmas_done:
                    nc.sync.dma_start(msk_sb[:], msk_d)
                    nc.sync.dma_start(ones_sb[:], ones_d)
                ctx_a.__exit__(None, None, None)
                # ---- phase B: attention (unnormalized) ------------------
                def attend_win(h, w):
                    if True:
                        ntk = (w + 1) * (TQW // P)

                        def qk_exp(i, h=h, w=w):
                            st = ps.tile([P, TQW], F32, tag="ps")
                            nc.tensor.matmul(
                                st[:],
                                k_t[h][i // (TQW // P)][:, bass.ts(i % (TQW // P), P)],
                                q_t[h][w][:],
                                start=True,
                                stop=True,
                            )
                            e = expp.tile([P, TQW], F32R, tag="exp")
                            nc.scalar.activation(e[:], st[:], AF.Exp, scale=scale)
                            off = i - w * (TQW // P)
                            if off >= 0:
                                nc.vector.tensor_tensor(
                                    e[:], e[:].bitcast(F32), msk_sb[:, off, :], op=MULT
                                )
                            return e

                        fifo = [qk_exp(j) for j in range(min(3, ntk))]
                        pv = ps.tile([P, TQW], F32, tag="ps")
                        den = ps.tile([P, TQW], F32, tag="ps")
                        for i in range(ntk):
                            if i + 3 < ntk:
                                fifo.append(qk_exp(i + 3))
                            e = fifo.pop(0)
                            nc.tensor.matmul(
                                pv[:],
                                v_sb[:, i, bass.ts(h, D)],
                                e[:],
                                start=(i == 0),
                                stop=(i == ntk - 1),
                            )
                            nc.tensor.matmul(
                                den[:1, :],
                                ones_sb[:],
                                e[:],
                                start=(i == 0),
                                stop=(i == ntk - 1),
                            )
                        # stash unnormalized attnT into the spent q window;
                        # den row -> bc tile row 0, replicate on gpsimd, then
                        # 1/x on DVE (2 custom ops, ~2 ULP, no ACT tables)
                        # and normalize the window in place.
                        nc.vector.tensor_copy(q_t[h][w][:], pv[:])
                        bc = bc_t[h][w]
                        nc.vector.tensor_copy(bc[:1, :], den[:1, :])
                        nc.gpsimd.partition_broadcast(bc[:], bc[:1, :])
                        scr = smallp.tile([P, TQW], F32, tag="t1", name="scr")
                        nc.vector.reciprocal_approx_accurate(
                            out=bc[:], in_=bc[:], scratch=scr[:]
                        )
                        nc.vector.tensor_tensor(
                            q_t[h][w][:], q_t[h][w][:].bitcast(F32), bc[:], op=MULT
                        )

                def phase_c_win(w, half=None):
                    ms = range(w * (TQW // P), (w + 1) * (TQW // P))
                    if half is not None:
                        ms = ms[: len(ms) // 2] if half == 0 else ms[len(ms) // 2 :]
                    for m in ms:
                        for n in range(C // TQW):
                            po = ps.tile([P, TQW], F32, tag="ps")
                            for h in range(HPC):
                                nc.tensor.matmul(
                                    po[:],
                                    q_t[h][m // (TQW // P)][
                                        :, bass.ts(m % (TQW // P), P)
                                    ],
                                    ow_sb[:, h, bass.ts(n, TQW)],
                                    start=(h == 0),
                                    stop=(h == HPC - 1),
                                )
                            o_t = outp.tile([P, TQW], F32, tag="o")
                            nc.any.tensor_copy(o_t[:], po[:])
                            nc.sync.dma_start(out_d[b, m, n], o_t[:])

                # ---- attention + output projection, software-pipelined:
                # phase C of window w-1 runs between attention windows so
                # the output DMA overlaps compute (only the last window's
                # store is tail-exposed).
                if not late_dmas_done:
                    nc.sync.dma_start(ow_sb[:], ow)
                    late_dmas_done.append(True)
                with nc.named_scope(f"BC{b}"):
                    for w in range(n_w):
                        attend_win(0, w)
                        if w > 0:
                            phase_c_win(w - 1, half=0)
                        attend_win(1, w)
                        if w > 0:
                            phase_c_win(w - 1, half=1)
                    phase_c_win(n_w - 1)

    nc.compile()
    return nc


def _host_prep(hidden_states, q_w, k_w, v_w, o_w):
    """Build the 8 per-core input maps (and shared constant tensors)."""
    T = hidden_states.shape[1]
    f32 = np.float32

    n_w = T // TQW
    KT = C // P
    # [B, T, C] -> hs^T blocked per (partition, window, k-quarter):
    # [B, P, n_w, KT//4, 4, TQW]
    hstT = hidden_states.transpose(0, 2, 1)  # [B, C, T]
    hst = np.ascontiguousarray(
        hstT.reshape(B, KT // 4, 4, P, n_w, TQW).transpose(0, 3, 4, 1, 2, 5)
    ).astype(f32, copy=False)

    def wblk(w_slice):
        # [DPC, C] row-slice -> w^T blocked [P, KT, DPC]
        return np.ascontiguousarray(
            w_slice.T.reshape(KT, P, DPC).transpose(1, 0, 2)
        ).astype(f32, copy=False)


    inv_freq = 1.0 / (ROPE_BASE ** (np.arange(0, D, 2, dtype=np.float64) / D))
    t_ar = np.arange(T, dtype=np.float64)
    freqs = t_ar[:, None] * inv_freq[None, :]  # [T, D/2]
    cos_td = np.concatenate([np.cos(freqs), np.cos(freqs)], axis=-1)  # [T, D]
    sin_td = np.concatenate([np.sin(freqs), np.sin(freqs)], axis=-1)
    cos_t = np.ascontiguousarray(cos_td.T).astype(f32)  # [D, T]
    sin_t = np.ascontiguousarray(sin_td.T).astype(f32)

    # rotate_half as a matmul: rh = R @ x ; rperm = R^T (lhsT operand).
    rperm = np.zeros((D, D), dtype=f32)
    half = D // 2
    for j in range(half):
        rperm[2 * j + 1, j] = -1.0
    for j in range(half, D):
        rperm[2 * (j - half), j] = 1.0

    ones = np.ones((P, 1), dtype=f32)

    n_off = TQW // P
    masks = np.zeros((P, n_off, TQW), dtype=f32)
    y = np.arange(P)[:, None]
    x = np.arange(TQW)[None, :]
    for o in range(n_off):
        masks[:, o, :] = (x >= P * o + y).astype(f32)

    in_maps = []
    for c in range(N_CORES):
        rs, re = c * DPC, (c + 1) * DPC
        in_maps.append(
            {
                "hst": hst,
                "wq_t": wblk(q_w[rs:re, :]),
                "wk_t": wblk(k_w[rs:re, :]),
                "wv_t": wblk(v_w[rs:re, :]),
                "ow_t": np.ascontiguousarray(o_w[:, rs:re].T.reshape(HPC, P, C).transpose(1, 0, 2)),
                "cos_t": cos_t,
                "sin_t": sin_t,
                "rperm": rperm,
                "ones": ones,
                "masks": masks,
            }
        )
    return in_maps


_NC_CACHE = {}


def _get_nc(T):
    if T not in _NC_CACHE:
        _NC_CACHE[T] = _build_nc(T)
    return _NC_CACHE[T]


def kernel(hidden_states, q_w, k_w, v_w, o_w, **run_kwargs):
    hidden_states = np.asarray(hidden_states, dtype=np.float32)
    q_w = np.asarray(q_w, dtype=np.float32)
    k_w = np.asarray(k_w, dtype=np.float32)
    v_w = np.asarray(v_w, dtype=np.float32)
    o_w = np.asarray(o_w, dtype=np.float32)
    T = hidden_states.shape[1]
    nc = _get_nc(T)
    in_maps = _host_prep(hidden_states, q_w, k_w, v_w, o_w)
    res = bass_utils.run_bass_kernel_spmd(
        nc, in_maps, core_ids=list(range(N_CORES)), **run_kwargs
    )
    out = np.zeros((B, T // P, C // TQW, P, TQW), dtype=np.float64)
    for r in res.results:
        out += r["out_p"].astype(np.float64)
    kernel.last_results = res
    return (
        out.transpose(0, 1, 3, 2, 4).reshape(B, T, C).astype(np.float32)
    )
